# revision 1
# baseline (speedup 1.0000x reference)
"""Trainium2 Bass kernel for nn_Loss_Synonymy.

Computes: sum over rows of relu(1 -/+ tanh(||S1_row - S2_row||_2)), the sign
chosen per-row by synonymy_score >= 0.6.

Strategy (pure data-parallel over 8 NeuronCores):
  - Shard the batch dim B=1048576 across 8 cores (131072 rows each).
  - Per core, stream [128, 4096] f32 slabs of S1/S2 (2 MiB DMAs; each
    partition holds 32 consecutive rows of 128 elems). DVE subtract,
    ACT square (in place), DVE segmented reduce over the innermost 128
    gives per-row sum-of-squares.
  - Epilogue on [128, 1024] buffers: sqrt -> tanh -> clamp to 1.0;
    score -> sign in {-1,+1} via is_lt(0.6); fused multiply+reduce gives
    per-partition partial sums of sign*tanh(dist).
  - Host: result = B + sum(all partials)   (since err = 1 + sign*t >= 0).
"""

import sys

if "/opt/trn_rl_repo" not in sys.path:
    sys.path.insert(0, "/opt/trn_rl_repo")

import numpy as np

B, D = 1048576, 128
NCORES = 8
BS = B // NCORES          # rows per core = 131072
P = 128                   # SBUF partitions
COLS = 4096               # free elems per slab
R = COLS // D             # rows per partition per slab = 32
NSLAB = BS // (P * R)     # slabs per core = 32
CPP = BS // P             # per-row values per partition = 1024
THRESH = 0.6

_nc_cache = {}


def _build_nc(reps=1, nslab=NSLAB, cols=COLS):
    """Build the per-core Bass program. reps>1 repeats the streaming main
    loop inside one NEFF (timing-measurement builds only). nslab/cols can be
    shrunk for debugging runs."""
    import concourse.bass as bass  # noqa: F401
    from concourse import bacc
    import concourse.tile as tile
    import concourse.mybir as mybir

    f32 = mybir.dt.float32
    bs = nslab * P * (cols // D)
    rr = cols // D
    cpp = bs // P
    # Bacc (not raw Bass): its compile() pass splits multi-sem waits onto
    # EventSemaphore carriers, required by TRN2's 1-wait-per-instruction limit.
    nc = bacc.Bacc(None)
    s1 = nc.dram_tensor("s1", [bs, D], f32, kind="ExternalInput")
    s2 = nc.dram_tensor("s2", [bs, D], f32, kind="ExternalInput")
    sc = nc.dram_tensor("score", [bs], f32, kind="ExternalInput")
    out = nc.dram_tensor("out", [P, 1], f32, kind="ExternalOutput")

    with tile.TileContext(nc) as tc:
        with (
            tc.tile_pool(name="p1", bufs=3) as p1,
            tc.tile_pool(name="p2", bufs=3) as p2,
            tc.tile_pool(name="psq", bufs=3) as psq,
            tc.tile_pool(name="pers", bufs=1) as pp,
        ):
            ss_all = pp.tile([P, cpp], f32)   # per-row sum-of-squares
            sc_all = pp.tile([P, cpp], f32)   # per-row synonymy score
            acc = pp.tile([P, 1], f32)

            s1v = s1[:].rearrange("(s p r) d -> s p (r d)", s=nslab, p=P, r=rr)
            s2v = s2[:].rearrange("(s p r) d -> s p (r d)", s=nslab, p=P, r=rr)
            scv = sc[:].rearrange("(s p r) -> p s r", s=nslab, p=P, r=rr)

            # One strided DMA brings the whole score shard into the layout
            # matching ss_all ([p, s*R + r] = row s*P*R + p*R + r).
            nc.sync.dma_start(
                sc_all[:].rearrange("p (s r) -> p s r", s=nslab, r=rr), scv
            )

            for _rep in range(reps):
                for s in range(nslab):
                    t1 = p1.tile([P, cols], f32)
                    nc.sync.dma_start(t1[:], s1v[s])
                    t2 = p2.tile([P, cols], f32)
                    nc.sync.dma_start(t2[:], s2v[s])
                    sq = psq.tile([P, cols], f32)
                    nc.vector.tensor_sub(sq[:], t1[:], t2[:])
                    nc.scalar.square(sq[:], sq[:])
                    nc.vector.reduce_sum(
                        ss_all[:, s * rr:(s + 1) * rr],
                        sq[:].rearrange("p (r d) -> p r d", d=D),
                        axis=mybir.AxisListType.X,
                    )

            # dist = sqrt(ss); t = tanh(dist); clamp t to <= 1.0 so that
            # relu(1 +/- t) == 1 +/- t exactly.
            nc.scalar.sqrt(ss_all[:], ss_all[:])
            nc.scalar.activation(
                ss_all[:], ss_all[:], mybir.ActivationFunctionType.Tanh
            )
            nc.vector.tensor_scalar_min(ss_all[:], ss_all[:], 1.0)
            # sign = +1 where score < 0.6, -1 where score >= 0.6:
            # (score is_lt 0.6) * 2 - 1
            nc.vector.tensor_scalar(
                sc_all[:], sc_all[:], THRESH, 2.0,
                op0=mybir.AluOpType.is_lt, op1=mybir.AluOpType.mult,
            )
            nc.vector.tensor_scalar_add(sc_all[:], sc_all[:], -1.0)
            # acc[p] = sum_c sign[p,c] * t[p,c]
            nc.vector.tensor_mul(sc_all[:], sc_all[:], ss_all[:])
            nc.vector.reduce_sum(acc[:], sc_all[:], axis=mybir.AxisListType.X)
            nc.sync.dma_start(out[:], acc[:])
    nc.finalize()
    return nc


def _get_nc(reps=1):
    if reps not in _nc_cache:
        _nc_cache[reps] = _build_nc(reps)
    return _nc_cache[reps]


def _in_maps(S1_out, S2_out, synonymy_score):
    s1 = np.ascontiguousarray(np.asarray(S1_out, dtype=np.float32))
    s2 = np.ascontiguousarray(np.asarray(S2_out, dtype=np.float32))
    sc = np.ascontiguousarray(np.asarray(synonymy_score, dtype=np.float32))
    assert s1.shape == (B, D) and s2.shape == (B, D) and sc.shape == (B,)
    return [
        {
            "s1": s1[c * BS:(c + 1) * BS],
            "s2": s2[c * BS:(c + 1) * BS],
            "score": sc[c * BS:(c + 1) * BS],
        }
        for c in range(NCORES)
    ]


def _postprocess(results):
    partials = np.concatenate([r["out"].ravel() for r in results])
    total = np.float64(B) + partials.astype(np.float64).sum()
    return np.float32(total)


def kernel(S1_out, S2_out, synonymy_score):
    from concourse.bass_utils import run_bass_kernel_spmd

    in_maps = _in_maps(S1_out, S2_out, synonymy_score)
    res = run_bass_kernel_spmd(_get_nc(), in_maps, list(range(NCORES)))
    return _postprocess(res.results)

